# revision 41
# baseline (speedup 1.0000x reference)
"""Trainium2 Bass kernel for nn_CausalSelfAttention_40810779247124.

Head-sharded (tensor-parallel) causal self-attention prefill across 8
NeuronCores: 2 heads per core.  fp8 DoubleRow projections + free
softmax denominator + software-pipelined emission.

Key ideas vs the bf16 baseline (320.6us -> 246.0us):

  * QKV and output projections run as fp8-e4m3 DoubleRow matmuls (0.5
    cycles/row).  Each operand is split hi/lo on the host (x = xh + xl,
    both e4m3, bf16-level combined accuracy) and the product computed
    with 3 DoubleRow terms: xh*wh paired across contraction chunks,
    plus one (wh*xl + wl*xh) cross DoubleRow per chunk -> 25% fewer PE
    cycles than bf16 at full accuracy.  Operands are pre-scaled (x*8,
    w*64) to keep the lo residuals out of the fp8 denormal range;
    compensation is folded into the exp scale, the denominator column,
    and a host-side divide of the partial sums.
  * Attention keeps scores [t,s] in bf16, but the PV matmul is flipped
    to produce wv^T [s,e] per 128-column quarter.  The softmax
    denominator becomes a free=1 matmul into column 128 of the same
    [128,129] psum tile (same accumulation group as the PV quarter --
    one start/stop pair per zero-region, since a start pending-zeroes
    the whole 2KB psum bank).  This replaces the baseline's free=512
    ones-row matmul (29us of PE).  Normalize is a per-partition
    tensor_scalar on DVE; the result is PE-transposed back to [e,s]
    (deferred to avoid in-order PE stalls) and split hi/lo into fp8 on
    ACT+DVE to feed the fp8 out-projection.
  * Everything is emitted as one software-pipelined stream: attention
    score-chunks, PV-quarters and out-projection psum groups are
    drip-fed between the QKV matmul groups as soon as their inputs are
    resident (batch 0 rides on the a=4..7 projection tiles, batch 1 on
    the tail), so the exp/drain work on ACT/DVE hides under the
    PE-bound projection and the in-order PE never sits behind a
    blocked dependent matmul.
  * Causal work is exact at 128-column granularity as in the baseline.

Per-core PE cycles: 295k (QKV) + 70k (scores) + 72k (PV+z) + 8k
(transposes) + 98k (out-proj) ~= 543k ~= 226us vs 733k/306us baseline;
measured total 246.0us (PE ~92% busy).

The host verifies mask/cache_pos match causal prefill and falls back to
a numpy reference otherwise.
"""

import sys

sys.path.insert(0, "/opt/trn_rl_repo")

import numpy as np

B = 2
S = 2048
T = 4096
NS = 2048          # n_state
H = 16
DH = 128
NCORES = 8
HPC = H // NCORES  # heads per core = 2
DPC = HPC * DH     # d-slice per core = 256
TOK = B * S        # 4096 tokens across batches
NT = TOK // 512    # 8 token tiles
NK = NS // 128     # 16 contraction chunks
SCALE = 1.0 / float(np.sqrt(DH))

AX = 8.0           # host pre-scale on x
AW = 64.0          # host pre-scale on w_qkv
AO = 64.0          # host pre-scale on w_out
ANWV = 8.0         # on-device scale of normalized wv (via the z column)
# z column value: wv carries AX*AW, so z must carry AX*AW/ANWV for the
# normalized wv to come out scaled by ANWV.
ZCOL = AX * AW / ANWV
# exp( SCALE * q.k ) with q,k carrying AX*AW each
ESCALE = SCALE / (AX * AW) ** 2
# out-projection partials carry ANWV * AO
OUT_SCALE = 1.0 / (ANWV * AO)

_CACHED = {}


def _build_program():
    import concourse.bacc as bacc
    import concourse.bass as bass
    import concourse.tile as tile
    from concourse import mybir

    bf16 = mybir.dt.bfloat16
    f32 = mybir.dt.float32
    fp8 = mybir.dt.float8e4
    DR = mybir.MatmulPerfMode.DoubleRow
    EXP = mybir.ActivationFunctionType.Exp
    COPY = mybir.ActivationFunctionType.Copy
    SUB = mybir.AluOpType.subtract

    nc = bacc.Bacc()

    # x hi/lo fp8, tiled: [NS, tile, (lo,hi), 512]
    xhl = nc.dram_tensor("xhl", [NS, NT, 2, 512], fp8, kind="ExternalInput")
    # w hi/lo fp8: [NS, (hi,lo), 768]  (q0,q1,k0,k1,v0,v1 columns)
    whl = nc.dram_tensor("whl", [NS, 2, 6 * DH], fp8, kind="ExternalInput")
    # w_out hi/lo fp8: [DPC, (hi,lo), NS]
    wouthl = nc.dram_tensor("wouthl", [DPC, 2, NS], fp8, kind="ExternalInput")
    # [tri | identity | zcol]
    cmask = nc.dram_tensor("cmask", [DH, 2 * DH + 1], bf16, kind="ExternalInput")
    outp = nc.dram_tensor("outp", [TOK, NS], bf16, kind="ExternalOutput")

    with tile.TileContext(nc) as tc:
        with (
            tc.tile_pool(name="constp", bufs=1) as constp,
            tc.tile_pool(name="vresp", bufs=1) as vresp,
            tc.tile_pool(name="qkresp", bufs=1) as qkresp,
            tc.tile_pool(name="woutp", bufs=1) as woutp,
            tc.tile_pool(name="ptp", bufs=24) as ptp,
            tc.tile_pool(name="zrp", bufs=4) as zrp,
            tc.tile_pool(name="nwvp", bufs=64) as nwvp,
            tc.tile_pool(name="wvnp", bufs=2) as wvnp,
            tc.tile_pool(name="ostage", bufs=3) as ostage,
            tc.tile_pool(name="sc_ps", bufs=2, space="PSUM") as sc_ps,
            tc.tile_pool(name="wvq_ps", bufs=2, space="PSUM") as wvq_ps,
            tc.tile_pool(name="tp_ps", bufs=1, space="PSUM") as tp_ps,
            tc.tile_pool(name="o_ps", bufs=1, space="PSUM") as o_ps,
        ):
            # tri[t, s] = 1.0 if s >= t; identity for PE transpose; zcol.
            tri2 = constp.tile([DH, 2 * DH + 1], bf16)
            tri = tri2[:, 0:DH]
            ident = tri2[:, DH : 2 * DH]
            zcol = tri2[:, 2 * DH : 2 * DH + 1]

            # V resident: v_res[p, c, e] = V[c*128+p, e] (tok-major)
            v_res = vresp.tile([128, TOK // 128, DPC], bf16)
            # Q,K resident [e-block(q0,q1,k0,k1), tok]
            qk_res = qkresp.tile([128, 4, TOK], bf16)
            wout_sb = woutp.tile([128, HPC, 2, NS], fp8)

            pending = []  # deferred PE transposes: (nwv, wvn_b, h, scol)
            state = {"tp": 0, "tp_all": None, "dq": 0, "op": 0, "opools": None}
            state["tp_all"] = tp_ps.tile([128, 8, 128], bf16, name="tp_all")
            state["opools"] = [o_ps]
            dqueues = [nc.sync, nc.sync]

            def flush_tp(n=None):
                cnt = len(pending) if n is None else min(n, len(pending))
                for _ in range(cnt):
                    nwv, wvn_b, h, scol = pending.pop(0)
                    r = state["tp"] % 8
                    state["tp"] += 1
                    tps = state["tp_all"][:, r, :]
                    nc.tensor.transpose(tps, nwv, ident)
                    hi = wvn_b[:, h, 1, scol : scol + 128]
                    nc.scalar.activation(out=hi, in_=tps, func=COPY, scale=1.0)
                    nc.vector.tensor_tensor(
                        out=wvn_b[:, h, 0, scol : scol + 128],
                        in0=tps,
                        in1=hi,
                        op=SUB,
                    )

            # Attention + out-projection work-units (one score-chunk, one
            # PV-quarter, or one out-proj psum group each), drip-fed between
            # QKV matmul groups so the in-order PE never sits behind a
            # blocked dependent matmul and the ACT/DVE load spreads over
            # the whole timeline.
            ready = []

            def make_tile(b, h, ast, wvn_b):
                q_sb = qk_res[:, h, S * b + 512 * ast : S * b + 512 * (ast + 1)]
                nfull = 4 * ast
                nj = nfull + 4
                pts = []

                def mk_chunk(j):
                    def emit():
                        p = j - nfull
                        lo = 0 if p < 0 else 128 * p  # causal narrowing
                        sc = sc_ps.tile([128, 512], f32, tag="sc", name="sc")
                        nc.tensor.matmul(
                            sc[:, lo:],
                            qk_res[
                                :, 2 + h, S * b + 128 * j : S * b + 128 * (j + 1)
                            ],
                            q_sb[:, lo:],
                            start=True,
                            stop=True,
                        )
                        pt = ptp.tile([128, 512], bf16, tag="pt", name="pt")
                        nc.scalar.activation(
                            out=pt[:, lo:], in_=sc[:, lo:], func=EXP, scale=ESCALE
                        )
                        if p >= 0:
                            nc.gpsimd.tensor_mul(
                                pt[:, lo : lo + 128], pt[:, lo : lo + 128], tri
                            )
                        pts.append(pt)

                    return emit

                def mk_quarter(q):
                    def emit():
                        # one psum accumulation group over one [128,129] tile
                        # (wv in cols 0..128, z in col 128 — one zero-region)
                        qs = slice(128 * q, 128 * (q + 1))
                        njq = nfull + q + 1
                        wz = wvq_ps.tile([128, 129], f32, tag="wvq", name="wz")
                        for j in range(njq):
                            nc.tensor.matmul(
                                wz[:, 0:128],
                                pts[j][:, qs],
                                v_res[:, 16 * b + j, 128 * h : 128 * (h + 1)],
                                start=(j == 0),
                                stop=False,
                                skip_group_check=True,
                            )
                            nc.tensor.matmul(
                                wz[:, 128:129],
                                pts[j][:, qs],
                                zcol,
                                start=False,
                                stop=(j == njq - 1),
                                skip_group_check=True,
                            )
                        # normalize [s,e] with a per-partition 1/z; transpose
                        # into [e,s] deferred (PE-stall avoidance).
                        zr = zrp.tile([128, 1], f32, tag="zr", name="zr")
                        nc.vector.reciprocal(out=zr, in_=wz[:, 128:129])
                        nwv = nwvp.tile([128, 128], bf16, tag="nwv", name="nwv")
                        nc.vector.tensor_scalar_mul(
                            nwv, wz[:, 0:128], zr[:, 0:1]
                        )
                        pending.append((nwv, wvn_b, h, 512 * ast + 128 * q))

                    return emit

                return [mk_chunk(j) for j in range(nj)] + [
                    mk_quarter(q) for q in range(4)
                ]

            def mk_ogroup(b, ast, wvn_b, tk, n, st, last=False):
                def emit():
                    toff = 512 * ast + 128 * tk
                    if n == 0:
                        st[tk] = ostage.tile(
                            [128, NS], bf16, tag="ost", name="ost"
                        )
                    ost = st[tk]
                    if last and n % 2:
                        # the last job runs with attention done: rotate
                        # through the idle score banks for extra depth
                        ops = sc_ps.tile([128, 512], f32, tag="sc", name="sc2")
                    else:
                        pools = state["opools"]
                        pool = pools[state["op"] % len(pools)]
                        state["op"] += 1
                        ops = pool.tile([128, 512], f32, tag="ops", name="ops")
                    nsl = slice(512 * n, 512 * (n + 1))
                    nc.tensor.matmul(
                        ops,
                        wvn_b[:, 0:2, 1, toff : toff + 128],
                        wout_sb[:, 0:2, 0, nsl],
                        start=True,
                        stop=False,
                        perf_mode=DR,
                    )
                    nc.tensor.matmul(
                        ops,
                        wvn_b[:, 0, 0:2, toff : toff + 128],
                        wout_sb[:, 0, 0:2, nsl],
                        start=False,
                        stop=False,
                        perf_mode=DR,
                    )
                    nc.tensor.matmul(
                        ops,
                        wvn_b[:, 1, 0:2, toff : toff + 128],
                        wout_sb[:, 1, 0:2, nsl],
                        start=False,
                        stop=True,
                        perf_mode=DR,
                    )
                    dst = ost[:, nsl]
                    if n % 2 == 0:
                        nc.vector.tensor_copy(out=dst, in_=ops)
                    else:
                        nc.scalar.activation(
                            out=dst, in_=ops, func=COPY, scale=1.0
                        )
                    if last:
                        # finer tail DMAs so the last transfer starts sooner
                        dq = dqueues[state["dq"] % 2]
                        state["dq"] += 1
                        dq.dma_start(
                            out=outp[S * b + toff : S * b + toff + 128, nsl],
                            in_=ost[:, nsl],
                        )
                    elif n % 2 == 1:
                        dq = dqueues[state["dq"] % 2]
                        state["dq"] += 1
                        dq.dma_start(
                            out=outp[
                                S * b + toff : S * b + toff + 128,
                                1024 * (n // 2) : 1024 * (n // 2 + 1),
                            ],
                            in_=ost[:, 1024 * (n // 2) : 1024 * (n // 2 + 1)],
                        )

                return emit

            def make_job(b, ast, wvn_b, last=False):
                st = {}
                units = [lambda: flush_tp(8)]
                for tk in range(4):
                    for n in range(4):
                        units.append(
                            mk_ogroup(b, ast, wvn_b, tk, n, st, last=last)
                        )
                return units

            def merge_push(tile_units, job_units):
                # round-robin so out-proj psum groups never emit
                # back-to-back (their drains need breathing room)
                if not job_units:
                    ready.extend(tile_units)
                    return
                ratio = max(1, len(tile_units) // len(job_units))
                ti = 0
                for ju in job_units:
                    ready.extend(tile_units[ti : ti + ratio])
                    ti += ratio
                    ready.append(ju)
                ready.extend(tile_units[ti:])

            def emit_units(n):
                for _ in range(min(n, len(ready))):
                    ready.pop(0)()

            # ---------------- phase 1: QKV projection (fp8 DoubleRow) ----
            # b=0 attention tiles ride along with the a=4..7 QKV tiles: the
            # exp/finalize work hides under the PE-bound projection.
            with (
                tc.tile_pool(name="wp", bufs=1) as wp,
                tc.tile_pool(name="xp", bufs=2) as xp,
                tc.tile_pool(name="mm_ps", bufs=2, space="PSUM") as mm_ps,
            ):
                nc.gpsimd.dma_start(out=tri2[:, :], in_=cmask[:, :])
                w_sb = wp.tile([128, NK, 2, 6 * DH], fp8)
                wvn0 = wvnp.tile([128, HPC, 2, S], fp8, tag="wvn", name="wvn0")
                wvn1 = wvnp.tile([128, HPC, 2, S], fp8, tag="wvn", name="wvn1")

                def qkv_group(a, x_sb, ps, mcols, flip):
                    """24-matmul accumulation group for one out-block.
                    flip=False: out [qkv-rows, tok]; True: V^T [tok, v-cols]."""
                    for kk in range(NK):
                        kc = kk
                        if not flip:
                            nc.tensor.matmul(
                                ps,
                                w_sb[:, kk, :, mcols],
                                x_sb[:, kc, :, :],
                                start=(kk == 0),
                                stop=False,
                                perf_mode=DR,
                            )
                        else:
                            nc.tensor.matmul(
                                ps,
                                x_sb[:, kc, :, mcols],
                                w_sb[:, kk, :, 512:768],
                                start=(kk == 0),
                                stop=False,
                                perf_mode=DR,
                            )
                        if kk % 2 == 1:
                            lastk = kk == NK - 1
                            if not flip:
                                nc.tensor.matmul(
                                    ps,
                                    w_sb[:, kk - 1 : kk + 1, 0, mcols],
                                    x_sb[:, kc - 1 : kc + 1, 1, :],
                                    start=False,
                                    stop=lastk,
                                    perf_mode=DR,
                                )
                            else:
                                nc.tensor.matmul(
                                    ps,
                                    x_sb[:, kc - 1 : kc + 1, 1, mcols],
                                    w_sb[:, kk - 1 : kk + 1, 0, 512:768],
                                    start=False,
                                    stop=lastk,
                                    perf_mode=DR,
                                )

                qs3 = [nc.sync, nc.gpsimd, nc.scalar]
                for a in range(NT):
                    x_sb = xp.tile([128, NK, 2, 512], fp8, tag="x_sb")
                    for kk in range(NK):
                        # round-robin dispatch across all three DMA queues
                        # so the a=0 prologue isn't dispatch-serialized
                        if a == 0:
                            xq = qs3[(2 * kk) % 3]
                            wq = qs3[(2 * kk + 1) % 3]
                            if kk == 0:
                                for mm in range(6):
                                    qs3[(mm + 1) % 3].dma_start(
                                        out=w_sb[:, kk, :, 128 * mm : 128 * (mm + 1)],
                                        in_=whl[
                                            128 * kk : 128 * (kk + 1),
                                            :,
                                            128 * mm : 128 * (mm + 1),
                                        ],
                                    )
                            else:
                                wq.dma_start(
                                    out=w_sb[:, kk, :, :],
                                    in_=whl[128 * kk : 128 * (kk + 1), :, :],
                                )
                        else:
                            # keep the gpsimd queue clear: the causal
                            # tri-mask multiplies live there and gate the
                            # PV matmuls
                            xq = nc.sync
                        xq.dma_start(
                            out=x_sb[:, kk, :, :],
                            in_=xhl[128 * kk : 128 * (kk + 1), a, :, :],
                        )
                    # q0,q1,k0,k1 blocks then V^T blocks, two psum tiles
                    # in flight; a few attention/out-proj units slip in
                    # after every group, plus one deferred transpose.
                    upg = 3 if a < 1 else (5 if a < 4 else (8 if a < 7 else 10))
                    for m in range(4):
                        ps = mm_ps.tile([128, 512], f32, tag="mm", name="mmq")
                        qkv_group(a, x_sb, ps, slice(128 * m, 128 * (m + 1)), False)
                        dst = qk_res[:, m, 512 * a : 512 * (a + 1)]
                        if m % 2 == 0:
                            nc.vector.tensor_copy(out=dst, in_=ps)
                        else:
                            nc.scalar.activation(
                                out=dst, in_=ps, func=COPY, scale=1.0
                            )
                        if a >= 1:
                            flush_tp(1)
                        emit_units(upg)
                    for t in range(4):
                        ps = mm_ps.tile([128, 256], f32, tag="mm", name="mmv")
                        qkv_group(a, x_sb, ps, slice(128 * t, 128 * (t + 1)), True)
                        dst = v_res[:, 4 * a + t, :]
                        if t % 2 == 0:
                            nc.vector.tensor_copy(out=dst, in_=ps)
                        else:
                            nc.scalar.activation(
                                out=dst, in_=ps, func=COPY, scale=1.0
                            )
                        if a >= 1:
                            flush_tp(1)
                        emit_units(upg)
                    if a == 0:
                        # Prefetch w_out and warm the ACT exp table while the
                        # PE grinds through the remaining QKV tiles.
                        for h in range(HPC):
                            nc.scalar.dma_start(
                                out=wout_sb[:, h, :, :],
                                in_=wouthl[128 * h : 128 * (h + 1), :, :],
                            )
                        warm = constp.tile([1, 1], f32)
                        nc.scalar.activation(
                            out=warm, in_=tri2[0:1, 0:1], func=EXP, scale=1.0
                        )
                    # attention tiles become ready as their QKV tiles drain
                    # (batch 0 after a=0..3, batch 1 after a=4..7); each
                    # pair's out-proj job is held one step and merged with
                    # the NEXT pair's units so its psum groups spread out.
                    nj_ = (0, a, wvn0) if a < 4 else (1, a - 4, wvn1)
                    tu = make_tile(nj_[0], 0, nj_[1], nj_[2]) + make_tile(
                        nj_[0], 1, nj_[1], nj_[2]
                    )
                    merge_push(tu, state.get("held") or [])
                    state["held"] = make_job(*nj_, last=(a == NT - 1))

            # ------- phase 2: the tail — b=1 ast=3 + its out-projection ---
            with (
                tc.tile_pool(name="o_ps2", bufs=2, space="PSUM") as o_ps2,
            ):
                state["opools"] = [o_ps, o_ps2, o_ps2]
                emit_units(len(ready))  # drain any leftover phase-1 units
                ready.extend(state.get("held") or [])  # job (b1, ast3)
                state["held"] = []
                emit_units(len(ready))
                flush_tp()

    nc.compile()
    return nc


def _causal_fastpath_ok(mask, cache_pos):
    if cache_pos.shape != (S,) or not np.array_equal(
        np.asarray(cache_pos), np.arange(S, dtype=np.int64).astype(cache_pos.dtype)
    ):
        return False
    m = np.asarray(mask).reshape(S, T)
    rows = np.arange(S)[:, None]
    cols = np.arange(T)[None, :]
    return np.array_equal(m, cols <= rows)


def _numpy_fallback(input_ids, mask, cache_pos, w_qkv, w_out, k_cache, v_cache):
    x = np.asarray(input_ids, dtype=np.float32)
    qkv = np.einsum("bsd,ed->bse", x, np.asarray(w_qkv, np.float32))
    q, k, v = np.split(qkv, 3, axis=-1)

    def heads(t):
        return t.reshape(B, S, H, DH).transpose(0, 2, 1, 3)

    q, k, v = heads(q), heads(k), heads(v)
    kf = np.array(k_cache, np.float32)
    vf = np.array(v_cache, np.float32)
    kf[:, :, np.asarray(cache_pos)] = k
    vf[:, :, np.asarray(cache_pos)] = v
    sc = np.einsum("bhsd,bhtd->bhst", q, kf) * SCALE
    sc = np.where(np.asarray(mask), sc, np.finfo(np.float32).min)
    sc = sc - sc.max(axis=-1, keepdims=True)
    p = np.exp(sc)
    p = p / p.sum(axis=-1, keepdims=True)
    wv = np.einsum("bhst,bhtd->bhsd", p, vf)
    wv = wv.transpose(0, 2, 1, 3).reshape(B, S, NS)
    return np.einsum("bsd,ed->bse", wv, np.asarray(w_out, np.float32))


def _build_cmask_host():
    # [tri | identity | zcol]: tri[t, s] = 1.0 if s >= t.
    t = np.arange(DH)[:, None]
    s = np.arange(DH)[None, :]
    tri = (s >= t).astype(np.float32)
    ident = np.eye(DH, dtype=np.float32)
    zc = np.full((DH, 1), ZCOL, np.float32)
    return np.concatenate([tri, ident, zc], axis=1)


def _run_on_device(in_maps, trace=False):
    from concourse.bass_utils import run_bass_kernel_spmd

    if "nc" not in _CACHED:
        _CACHED["nc"] = _build_program()
    nc = _CACHED["nc"]
    return run_bass_kernel_spmd(
        nc, in_maps, core_ids=list(range(NCORES)), trace=trace
    )


def _split_hl(arr32):
    """fp8 hi/lo split: arr32 ~= hi + lo with hi,lo e4m3."""
    import ml_dtypes

    f8 = ml_dtypes.float8_e4m3
    hi = arr32.astype(f8)
    lo = (arr32 - hi.astype(np.float32)).astype(f8)
    return hi, lo


def _prep_in_maps(input_ids, w_qkv, w_out):
    import ml_dtypes

    bf = ml_dtypes.bfloat16
    x2d = np.ascontiguousarray(
        np.asarray(input_ids, np.float32).reshape(TOK, NS).T
    ) * AX  # [NS, TOK], pre-scaled
    xh, xl = _split_hl(x2d)
    xhl = np.ascontiguousarray(
        np.stack([xl.reshape(NS, NT, 512), xh.reshape(NS, NT, 512)], axis=2)
    )  # [NS, NT, 2(lo,hi), 512]
    cm = _build_cmask_host().astype(bf)
    wq = np.asarray(w_qkv, np.float32)
    wo = np.asarray(w_out, np.float32)
    in_maps = []
    for c in range(NCORES):
        lo_, hi_ = c * DPC, (c + 1) * DPC
        w_slice = np.concatenate(
            [wq[lo_:hi_], wq[NS + lo_ : NS + hi_], wq[2 * NS + lo_ : 2 * NS + hi_]],
            axis=0,
        )  # [768, NS] (q,k,v rows for this core's heads)
        wT_c = np.ascontiguousarray(w_slice.T) * AW       # [NS, 768]
        wh, wl = _split_hl(wT_c)
        whl_c = np.ascontiguousarray(np.stack([wh, wl], axis=1))  # (hi,lo)
        woT_c = np.ascontiguousarray(wo[:, lo_:hi_].T) * AO  # [DPC, NS]
        woh, wol = _split_hl(woT_c)
        wouthl_c = np.ascontiguousarray(np.stack([woh, wol], axis=1))
        in_maps.append(
            {"xhl": xhl, "whl": whl_c, "wouthl": wouthl_c, "cmask": cm}
        )
    return in_maps


def kernel(input_ids, mask, cache_pos, w_qkv, w_out, k_cache, v_cache):
    if not _causal_fastpath_ok(mask, cache_pos):
        return _numpy_fallback(
            input_ids, mask, cache_pos, w_qkv, w_out, k_cache, v_cache
        )
    in_maps = _prep_in_maps(input_ids, w_qkv, w_out)
    res = _run_on_device(in_maps)
    out = np.zeros((TOK, NS), np.float32)
    for r in res.results:
        out += np.asarray(r["outp"], dtype=np.float32)
    out *= OUT_SCALE
    return out.reshape(B, S, NS)


# revision 42
# speedup vs baseline: 1.0087x; 1.0087x over previous
"""Trainium2 Bass kernel for nn_CausalSelfAttention_40810779247124.

Head-sharded (tensor-parallel) causal self-attention prefill across 8
NeuronCores: 2 heads per core.  fp8 DoubleRow projections + free
softmax denominator + software-pipelined emission.

Key ideas vs the bf16 baseline (320.6us -> 246.0us):

  * QKV and output projections run as fp8-e4m3 DoubleRow matmuls (0.5
    cycles/row).  Each operand is split hi/lo on the host (x = xh + xl,
    both e4m3, bf16-level combined accuracy) and the product computed
    with 3 DoubleRow terms: xh*wh paired across contraction chunks,
    plus one (wh*xl + wl*xh) cross DoubleRow per chunk -> 25% fewer PE
    cycles than bf16 at full accuracy.  Operands are pre-scaled (x*8,
    w*64) to keep the lo residuals out of the fp8 denormal range;
    compensation is folded into the exp scale, the denominator column,
    and a host-side divide of the partial sums.
  * Attention keeps scores [t,s] in bf16, but the PV matmul is flipped
    to produce wv^T [s,e] per 128-column quarter.  The softmax
    denominator becomes a free=1 matmul into column 128 of the same
    [128,129] psum tile (same accumulation group as the PV quarter --
    one start/stop pair per zero-region, since a start pending-zeroes
    the whole 2KB psum bank).  This replaces the baseline's free=512
    ones-row matmul (29us of PE).  Normalize is a per-partition
    tensor_scalar on DVE; the result is PE-transposed back to [e,s]
    (deferred to avoid in-order PE stalls) and split hi/lo into fp8 on
    ACT+DVE to feed the fp8 out-projection.
  * Everything is emitted as one software-pipelined stream: attention
    score-chunks, PV-quarters and out-projection psum groups are
    drip-fed between the QKV matmul groups as soon as their inputs are
    resident (batch 0 rides on the a=4..7 projection tiles, batch 1 on
    the tail), so the exp/drain work on ACT/DVE hides under the
    PE-bound projection and the in-order PE never sits behind a
    blocked dependent matmul.
  * Causal work is exact at 128-column granularity as in the baseline.

Per-core PE cycles: 295k (QKV) + 70k (scores) + 72k (PV+z) + 8k
(transposes) + 98k (out-proj) ~= 543k ~= 226us vs 733k/306us baseline;
measured total 246.0us (PE ~92% busy).

The host verifies mask/cache_pos match causal prefill and falls back to
a numpy reference otherwise.
"""

import sys

sys.path.insert(0, "/opt/trn_rl_repo")

import numpy as np

B = 2
S = 2048
T = 4096
NS = 2048          # n_state
H = 16
DH = 128
NCORES = 8
HPC = H // NCORES  # heads per core = 2
DPC = HPC * DH     # d-slice per core = 256
TOK = B * S        # 4096 tokens across batches
NT = TOK // 512    # 8 token tiles
NK = NS // 128     # 16 contraction chunks
SCALE = 1.0 / float(np.sqrt(DH))

AX = 8.0           # host pre-scale on x
AW = 64.0          # host pre-scale on w_qkv
AO = 64.0          # host pre-scale on w_out
ANWV = 8.0         # on-device scale of normalized wv (via the z column)
# z column value: wv carries AX*AW, so z must carry AX*AW/ANWV for the
# normalized wv to come out scaled by ANWV.
ZCOL = AX * AW / ANWV
# exp( SCALE * q.k ) with q,k carrying AX*AW each
ESCALE = SCALE / (AX * AW) ** 2
# out-projection partials carry ANWV * AO
OUT_SCALE = 1.0 / (ANWV * AO)

_CACHED = {}


def _build_program():
    import concourse.bacc as bacc
    import concourse.bass as bass
    import concourse.tile as tile
    from concourse import mybir

    bf16 = mybir.dt.bfloat16
    f32 = mybir.dt.float32
    fp8 = mybir.dt.float8e4
    DR = mybir.MatmulPerfMode.DoubleRow
    EXP = mybir.ActivationFunctionType.Exp
    COPY = mybir.ActivationFunctionType.Copy
    SUB = mybir.AluOpType.subtract

    nc = bacc.Bacc()

    # x hi/lo fp8, tiled: [NS, tile, (lo,hi), 512]
    xhl = nc.dram_tensor("xhl", [NS, NT, 2, 512], fp8, kind="ExternalInput")
    # w hi/lo fp8: [NS, (hi,lo), 768]  (q0,q1,k0,k1,v0,v1 columns)
    whl = nc.dram_tensor("whl", [NS, 2, 6 * DH], fp8, kind="ExternalInput")
    # w_out hi/lo fp8: [DPC, (hi,lo), NS]
    wouthl = nc.dram_tensor("wouthl", [DPC, 2, NS], fp8, kind="ExternalInput")
    # [tri | identity | zcol]
    cmask = nc.dram_tensor("cmask", [DH, 2 * DH + 1], bf16, kind="ExternalInput")
    outp = nc.dram_tensor("outp", [TOK, NS], bf16, kind="ExternalOutput")

    with tile.TileContext(nc) as tc:
        with (
            tc.tile_pool(name="constp", bufs=1) as constp,
            tc.tile_pool(name="vresp", bufs=1) as vresp,
            tc.tile_pool(name="qkresp", bufs=1) as qkresp,
            tc.tile_pool(name="woutp", bufs=1) as woutp,
            tc.tile_pool(name="ptp", bufs=24) as ptp,
            tc.tile_pool(name="zrp", bufs=4) as zrp,
            tc.tile_pool(name="nwvp", bufs=64) as nwvp,
            tc.tile_pool(name="wvnp", bufs=2) as wvnp,
            tc.tile_pool(name="ostage", bufs=3) as ostage,
            tc.tile_pool(name="sc_ps", bufs=2, space="PSUM") as sc_ps,
            tc.tile_pool(name="wvq_ps", bufs=2, space="PSUM") as wvq_ps,
            tc.tile_pool(name="tp_ps", bufs=1, space="PSUM") as tp_ps,
            tc.tile_pool(name="o_ps", bufs=1, space="PSUM") as o_ps,
        ):
            # tri[t, s] = 1.0 if s >= t; identity for PE transpose; zcol.
            tri2 = constp.tile([DH, 2 * DH + 1], bf16)
            tri = tri2[:, 0:DH]
            ident = tri2[:, DH : 2 * DH]
            zcol = tri2[:, 2 * DH : 2 * DH + 1]

            # V resident: v_res[p, c, e] = V[c*128+p, e] (tok-major)
            v_res = vresp.tile([128, TOK // 128, DPC], bf16)
            # Q,K resident [e-block(q0,q1,k0,k1), tok]
            qk_res = qkresp.tile([128, 4, TOK], bf16)
            wout_sb = woutp.tile([128, HPC, 2, NS], fp8)

            pending = []  # deferred PE transposes: (nwv, wvn_b, h, scol)
            state = {"tp": 0, "tp_all": None, "dq": 0, "op": 0, "opools": None}
            state["tp_all"] = tp_ps.tile([128, 8, 128], bf16, name="tp_all")
            state["opools"] = [o_ps]
            dqueues = [nc.sync, nc.gpsimd]

            def flush_tp(n=None):
                cnt = len(pending) if n is None else min(n, len(pending))
                for _ in range(cnt):
                    nwv, wvn_b, h, scol = pending.pop(0)
                    r = state["tp"] % 8
                    state["tp"] += 1
                    tps = state["tp_all"][:, r, :]
                    nc.tensor.transpose(tps, nwv, ident)
                    hi = wvn_b[:, h, 1, scol : scol + 128]
                    nc.scalar.activation(out=hi, in_=tps, func=COPY, scale=1.0)
                    nc.vector.tensor_tensor(
                        out=wvn_b[:, h, 0, scol : scol + 128],
                        in0=tps,
                        in1=hi,
                        op=SUB,
                    )

            # Attention + out-projection work-units (one score-chunk, one
            # PV-quarter, or one out-proj psum group each), drip-fed between
            # QKV matmul groups so the in-order PE never sits behind a
            # blocked dependent matmul and the ACT/DVE load spreads over
            # the whole timeline.
            ready = []

            def make_tile(b, h, ast, wvn_b):
                q_sb = qk_res[:, h, S * b + 512 * ast : S * b + 512 * (ast + 1)]
                nfull = 4 * ast
                nj = nfull + 4
                pts = []

                def mk_chunk(j):
                    def emit():
                        p = j - nfull
                        lo = 0 if p < 0 else 128 * p  # causal narrowing
                        sc = sc_ps.tile([128, 512], f32, tag="sc", name="sc")
                        nc.tensor.matmul(
                            sc[:, lo:],
                            qk_res[
                                :, 2 + h, S * b + 128 * j : S * b + 128 * (j + 1)
                            ],
                            q_sb[:, lo:],
                            start=True,
                            stop=True,
                        )
                        pt = ptp.tile([128, 512], bf16, tag="pt", name="pt")
                        nc.scalar.activation(
                            out=pt[:, lo:], in_=sc[:, lo:], func=EXP, scale=ESCALE
                        )
                        if p >= 0:
                            nc.gpsimd.tensor_mul(
                                pt[:, lo : lo + 128], pt[:, lo : lo + 128], tri
                            )
                        pts.append(pt)

                    return emit

                def mk_quarter(q):
                    def emit():
                        # one psum accumulation group over one [128,129] tile
                        # (wv in cols 0..128, z in col 128 — one zero-region)
                        qs = slice(128 * q, 128 * (q + 1))
                        njq = nfull + q + 1
                        wz = wvq_ps.tile([128, 129], f32, tag="wvq", name="wz")
                        for j in range(njq):
                            nc.tensor.matmul(
                                wz[:, 0:128],
                                pts[j][:, qs],
                                v_res[:, 16 * b + j, 128 * h : 128 * (h + 1)],
                                start=(j == 0),
                                stop=False,
                                skip_group_check=True,
                            )
                            nc.tensor.matmul(
                                wz[:, 128:129],
                                pts[j][:, qs],
                                zcol,
                                start=False,
                                stop=(j == njq - 1),
                                skip_group_check=True,
                            )
                        # normalize [s,e] with a per-partition 1/z; transpose
                        # into [e,s] deferred (PE-stall avoidance).
                        zr = zrp.tile([128, 1], f32, tag="zr", name="zr")
                        nc.vector.reciprocal(out=zr, in_=wz[:, 128:129])
                        nwv = nwvp.tile([128, 128], bf16, tag="nwv", name="nwv")
                        nc.vector.tensor_scalar_mul(
                            nwv, wz[:, 0:128], zr[:, 0:1]
                        )
                        pending.append((nwv, wvn_b, h, 512 * ast + 128 * q))

                    return emit

                return [mk_chunk(j) for j in range(nj)] + [
                    mk_quarter(q) for q in range(4)
                ]

            def mk_ogroup(b, ast, wvn_b, tk, n, st, last=False):
                def emit():
                    toff = 512 * ast + 128 * tk
                    if n == 0:
                        st[tk] = ostage.tile(
                            [128, NS], bf16, tag="ost", name="ost"
                        )
                    ost = st[tk]
                    if last and n % 2:
                        # the last job runs with attention done: rotate
                        # through the idle score banks for extra depth
                        ops = sc_ps.tile([128, 512], f32, tag="sc", name="sc2")
                    else:
                        pools = state["opools"]
                        pool = pools[state["op"] % len(pools)]
                        state["op"] += 1
                        ops = pool.tile([128, 512], f32, tag="ops", name="ops")
                    nsl = slice(512 * n, 512 * (n + 1))
                    nc.tensor.matmul(
                        ops,
                        wvn_b[:, 0:2, 1, toff : toff + 128],
                        wout_sb[:, 0:2, 0, nsl],
                        start=True,
                        stop=False,
                        perf_mode=DR,
                    )
                    nc.tensor.matmul(
                        ops,
                        wvn_b[:, 0, 0:2, toff : toff + 128],
                        wout_sb[:, 0, 0:2, nsl],
                        start=False,
                        stop=False,
                        perf_mode=DR,
                    )
                    nc.tensor.matmul(
                        ops,
                        wvn_b[:, 1, 0:2, toff : toff + 128],
                        wout_sb[:, 1, 0:2, nsl],
                        start=False,
                        stop=True,
                        perf_mode=DR,
                    )
                    dst = ost[:, nsl]
                    if n % 2 == 0:
                        nc.vector.tensor_copy(out=dst, in_=ops)
                    else:
                        nc.scalar.activation(
                            out=dst, in_=ops, func=COPY, scale=1.0
                        )
                    if last:
                        # finer tail DMAs so the last transfer starts sooner
                        dq = dqueues[state["dq"] % 2]
                        state["dq"] += 1
                        dq.dma_start(
                            out=outp[S * b + toff : S * b + toff + 128, nsl],
                            in_=ost[:, nsl],
                        )
                    elif n % 2 == 1:
                        dq = dqueues[state["dq"] % 2]
                        state["dq"] += 1
                        dq.dma_start(
                            out=outp[
                                S * b + toff : S * b + toff + 128,
                                1024 * (n // 2) : 1024 * (n // 2 + 1),
                            ],
                            in_=ost[:, 1024 * (n // 2) : 1024 * (n // 2 + 1)],
                        )

                return emit

            def make_job(b, ast, wvn_b, last=False):
                st = {}
                units = [lambda: flush_tp(8)]
                for tk in range(4):
                    for n in range(4):
                        units.append(
                            mk_ogroup(b, ast, wvn_b, tk, n, st, last=last)
                        )
                return units

            def merge_push(tile_units, job_units):
                # round-robin so out-proj psum groups never emit
                # back-to-back (their drains need breathing room)
                if not job_units:
                    ready.extend(tile_units)
                    return
                ratio = max(1, len(tile_units) // len(job_units))
                ti = 0
                for ju in job_units:
                    ready.extend(tile_units[ti : ti + ratio])
                    ti += ratio
                    ready.append(ju)
                ready.extend(tile_units[ti:])

            def emit_units(n):
                for _ in range(min(n, len(ready))):
                    ready.pop(0)()

            # ---------------- phase 1: QKV projection (fp8 DoubleRow) ----
            # b=0 attention tiles ride along with the a=4..7 QKV tiles: the
            # exp/finalize work hides under the PE-bound projection.
            with (
                tc.tile_pool(name="wp", bufs=1) as wp,
                tc.tile_pool(name="xp", bufs=2) as xp,
                tc.tile_pool(name="mm_ps", bufs=2, space="PSUM") as mm_ps,
            ):
                nc.gpsimd.dma_start(out=tri2[:, :], in_=cmask[:, :])
                w_sb = wp.tile([128, NK, 2, 6 * DH], fp8)
                wvn0 = wvnp.tile([128, HPC, 2, S], fp8, tag="wvn", name="wvn0")
                wvn1 = wvnp.tile([128, HPC, 2, S], fp8, tag="wvn", name="wvn1")

                def qkv_group(a, x_sb, ps, mcols, flip):
                    """24-matmul accumulation group for one out-block.
                    flip=False: out [qkv-rows, tok]; True: V^T [tok, v-cols]."""
                    for kk in range(NK):
                        kc = kk
                        if not flip:
                            nc.tensor.matmul(
                                ps,
                                w_sb[:, kk, :, mcols],
                                x_sb[:, kc, :, :],
                                start=(kk == 0),
                                stop=False,
                                perf_mode=DR,
                            )
                        else:
                            nc.tensor.matmul(
                                ps,
                                x_sb[:, kc, :, mcols],
                                w_sb[:, kk, :, 512:768],
                                start=(kk == 0),
                                stop=False,
                                perf_mode=DR,
                            )
                        if kk % 2 == 1:
                            lastk = kk == NK - 1
                            if not flip:
                                nc.tensor.matmul(
                                    ps,
                                    w_sb[:, kk - 1 : kk + 1, 0, mcols],
                                    x_sb[:, kc - 1 : kc + 1, 1, :],
                                    start=False,
                                    stop=lastk,
                                    perf_mode=DR,
                                )
                            else:
                                nc.tensor.matmul(
                                    ps,
                                    x_sb[:, kc - 1 : kc + 1, 1, mcols],
                                    w_sb[:, kk - 1 : kk + 1, 0, 512:768],
                                    start=False,
                                    stop=lastk,
                                    perf_mode=DR,
                                )

                qs3 = [nc.sync, nc.gpsimd, nc.scalar]
                for a in range(NT):
                    x_sb = xp.tile([128, NK, 2, 512], fp8, tag="x_sb")
                    for kk in range(NK):
                        # round-robin dispatch across all three DMA queues
                        # so the a=0 prologue isn't dispatch-serialized
                        if a == 0:
                            xq = qs3[(2 * kk) % 3]
                            wq = qs3[(2 * kk + 1) % 3]
                            if kk == 0:
                                for mm in range(6):
                                    qs3[(mm + 1) % 3].dma_start(
                                        out=w_sb[:, kk, :, 128 * mm : 128 * (mm + 1)],
                                        in_=whl[
                                            128 * kk : 128 * (kk + 1),
                                            :,
                                            128 * mm : 128 * (mm + 1),
                                        ],
                                    )
                            else:
                                wq.dma_start(
                                    out=w_sb[:, kk, :, :],
                                    in_=whl[128 * kk : 128 * (kk + 1), :, :],
                                )
                        else:
                            xq = nc.sync if kk % 2 == 0 else nc.gpsimd
                        xq.dma_start(
                            out=x_sb[:, kk, :, :],
                            in_=xhl[128 * kk : 128 * (kk + 1), a, :, :],
                        )
                    # q0,q1,k0,k1 blocks then V^T blocks, two psum tiles
                    # in flight; a few attention/out-proj units slip in
                    # after every group, plus one deferred transpose.
                    upg = 3 if a < 1 else (5 if a < 4 else (8 if a < 7 else 10))
                    for m in range(4):
                        ps = mm_ps.tile([128, 512], f32, tag="mm", name="mmq")
                        qkv_group(a, x_sb, ps, slice(128 * m, 128 * (m + 1)), False)
                        dst = qk_res[:, m, 512 * a : 512 * (a + 1)]
                        if m % 2 == 0:
                            nc.vector.tensor_copy(out=dst, in_=ps)
                        else:
                            nc.scalar.activation(
                                out=dst, in_=ps, func=COPY, scale=1.0
                            )
                        if a >= 1:
                            flush_tp(1)
                        emit_units(upg)
                    for t in range(4):
                        ps = mm_ps.tile([128, 256], f32, tag="mm", name="mmv")
                        qkv_group(a, x_sb, ps, slice(128 * t, 128 * (t + 1)), True)
                        dst = v_res[:, 4 * a + t, :]
                        if t % 2 == 0:
                            nc.vector.tensor_copy(out=dst, in_=ps)
                        else:
                            nc.scalar.activation(
                                out=dst, in_=ps, func=COPY, scale=1.0
                            )
                        if a >= 1:
                            flush_tp(1)
                        emit_units(upg)
                    if a == 0:
                        # Prefetch w_out and warm the ACT exp table while the
                        # PE grinds through the remaining QKV tiles.
                        for h in range(HPC):
                            nc.scalar.dma_start(
                                out=wout_sb[:, h, :, :],
                                in_=wouthl[128 * h : 128 * (h + 1), :, :],
                            )
                        warm = constp.tile([1, 1], f32)
                        nc.scalar.activation(
                            out=warm, in_=tri2[0:1, 0:1], func=EXP, scale=1.0
                        )
                    # attention tiles become ready as their QKV tiles drain
                    # (batch 0 after a=0..3, batch 1 after a=4..7); each
                    # pair's out-proj job is held one step and merged with
                    # the NEXT pair's units so its psum groups spread out.
                    nj_ = (0, a, wvn0) if a < 4 else (1, a - 4, wvn1)
                    tu = make_tile(nj_[0], 0, nj_[1], nj_[2]) + make_tile(
                        nj_[0], 1, nj_[1], nj_[2]
                    )
                    merge_push(tu, state.get("held") or [])
                    state["held"] = make_job(*nj_, last=(a == NT - 1))

            # ------- phase 2: the tail — b=1 ast=3 + its out-projection ---
            with (
                tc.tile_pool(name="o_ps2", bufs=2, space="PSUM") as o_ps2,
            ):
                state["opools"] = [o_ps, o_ps2, o_ps2]
                emit_units(len(ready))  # drain any leftover phase-1 units
                ready.extend(state.get("held") or [])  # job (b1, ast3)
                state["held"] = []
                emit_units(len(ready))
                flush_tp()

    nc.compile()
    return nc


def _causal_fastpath_ok(mask, cache_pos):
    if cache_pos.shape != (S,) or not np.array_equal(
        np.asarray(cache_pos), np.arange(S, dtype=np.int64).astype(cache_pos.dtype)
    ):
        return False
    m = np.asarray(mask).reshape(S, T)
    rows = np.arange(S)[:, None]
    cols = np.arange(T)[None, :]
    return np.array_equal(m, cols <= rows)


def _numpy_fallback(input_ids, mask, cache_pos, w_qkv, w_out, k_cache, v_cache):
    x = np.asarray(input_ids, dtype=np.float32)
    qkv = np.einsum("bsd,ed->bse", x, np.asarray(w_qkv, np.float32))
    q, k, v = np.split(qkv, 3, axis=-1)

    def heads(t):
        return t.reshape(B, S, H, DH).transpose(0, 2, 1, 3)

    q, k, v = heads(q), heads(k), heads(v)
    kf = np.array(k_cache, np.float32)
    vf = np.array(v_cache, np.float32)
    kf[:, :, np.asarray(cache_pos)] = k
    vf[:, :, np.asarray(cache_pos)] = v
    sc = np.einsum("bhsd,bhtd->bhst", q, kf) * SCALE
    sc = np.where(np.asarray(mask), sc, np.finfo(np.float32).min)
    sc = sc - sc.max(axis=-1, keepdims=True)
    p = np.exp(sc)
    p = p / p.sum(axis=-1, keepdims=True)
    wv = np.einsum("bhst,bhtd->bhsd", p, vf)
    wv = wv.transpose(0, 2, 1, 3).reshape(B, S, NS)
    return np.einsum("bsd,ed->bse", wv, np.asarray(w_out, np.float32))


def _build_cmask_host():
    # [tri | identity | zcol]: tri[t, s] = 1.0 if s >= t.
    t = np.arange(DH)[:, None]
    s = np.arange(DH)[None, :]
    tri = (s >= t).astype(np.float32)
    ident = np.eye(DH, dtype=np.float32)
    zc = np.full((DH, 1), ZCOL, np.float32)
    return np.concatenate([tri, ident, zc], axis=1)


def _run_on_device(in_maps, trace=False):
    from concourse.bass_utils import run_bass_kernel_spmd

    if "nc" not in _CACHED:
        _CACHED["nc"] = _build_program()
    nc = _CACHED["nc"]
    return run_bass_kernel_spmd(
        nc, in_maps, core_ids=list(range(NCORES)), trace=trace
    )


def _split_hl(arr32):
    """fp8 hi/lo split: arr32 ~= hi + lo with hi,lo e4m3."""
    import ml_dtypes

    f8 = ml_dtypes.float8_e4m3
    hi = arr32.astype(f8)
    lo = (arr32 - hi.astype(np.float32)).astype(f8)
    return hi, lo


def _prep_in_maps(input_ids, w_qkv, w_out):
    import ml_dtypes

    bf = ml_dtypes.bfloat16
    x2d = np.ascontiguousarray(
        np.asarray(input_ids, np.float32).reshape(TOK, NS).T
    ) * AX  # [NS, TOK], pre-scaled
    xh, xl = _split_hl(x2d)
    xhl = np.ascontiguousarray(
        np.stack([xl.reshape(NS, NT, 512), xh.reshape(NS, NT, 512)], axis=2)
    )  # [NS, NT, 2(lo,hi), 512]
    cm = _build_cmask_host().astype(bf)
    wq = np.asarray(w_qkv, np.float32)
    wo = np.asarray(w_out, np.float32)
    in_maps = []
    for c in range(NCORES):
        lo_, hi_ = c * DPC, (c + 1) * DPC
        w_slice = np.concatenate(
            [wq[lo_:hi_], wq[NS + lo_ : NS + hi_], wq[2 * NS + lo_ : 2 * NS + hi_]],
            axis=0,
        )  # [768, NS] (q,k,v rows for this core's heads)
        wT_c = np.ascontiguousarray(w_slice.T) * AW       # [NS, 768]
        wh, wl = _split_hl(wT_c)
        whl_c = np.ascontiguousarray(np.stack([wh, wl], axis=1))  # (hi,lo)
        woT_c = np.ascontiguousarray(wo[:, lo_:hi_].T) * AO  # [DPC, NS]
        woh, wol = _split_hl(woT_c)
        wouthl_c = np.ascontiguousarray(np.stack([woh, wol], axis=1))
        in_maps.append(
            {"xhl": xhl, "whl": whl_c, "wouthl": wouthl_c, "cmask": cm}
        )
    return in_maps


def kernel(input_ids, mask, cache_pos, w_qkv, w_out, k_cache, v_cache):
    if not _causal_fastpath_ok(mask, cache_pos):
        return _numpy_fallback(
            input_ids, mask, cache_pos, w_qkv, w_out, k_cache, v_cache
        )
    in_maps = _prep_in_maps(input_ids, w_qkv, w_out)
    res = _run_on_device(in_maps)
    out = np.zeros((TOK, NS), np.float32)
    for r in res.results:
        out += np.asarray(r["outp"], dtype=np.float32)
    out *= OUT_SCALE
    return out.reshape(B, S, NS)
